# revision 1
# baseline (speedup 1.0000x reference)
"""Trainium2 Bass kernel for nn_Dnn_with_Attention (ragged attention-pooled DNN).

Contract: kernel(**inputs) takes FULL unsharded numpy inputs (keys as in
reference.setup_inputs()) and returns the FULL [256, 10] float32 output.

Strategy (data-parallel over utterances, 8 NeuronCores):
  - Host: greedily balance the 256 segments over 8 cores (32 whole segments
    each), gather each core's frames, transpose x to feature-major
    [128(feat-padded), M_PAD] and build a per-frame one-hot segment
    membership matrix A [M_PAD/128, 128, 32].  A row of ones is appended as
    feature 78 so b1 folds into W1.
  - Device (per core): 4-layer MLP with activations kept feature-major
    (hT [1024, frames]) for layers 1-3; layer 4 produces frame-major
    h4 [128fr, 1024] (lhsT = h3T).  Scores via a DVE multiply + reduce
    against a replicated W5; e = exp(score) with the relu
    folded as max(e, 1).  Segment softmax pooling is done as small PE
    matmuls E.T @ h4 (E = A * e) accumulated into persistent PSUM across
    all chunks; the softmax denominator comes from E.T @ ones.  The final
    per-utterance MLP runs once at the end (W6 is DMA'd late into W4's
    SBUF slot to fit).
  - All matmuls use float32r (full-rate fp32 on the PE array); every
    matmul operand tile is typed float32r end-to-end to satisfy the
    walrus rounding rule.
"""

import sys

sys.path.insert(0, "/opt/trn_rl_repo")

import numpy as np

import concourse.bass as bass
import concourse.mybir as mybir
import concourse.tile as tile
from concourse import bacc
from concourse.bass_utils import run_bass_kernel_spmd

P = 128
FEAT = 78
HID = 1024
NCLS = 10
NSEG = 256
NCORES = 8
SEGS_PER_CORE = NSEG // NCORES
CH = 256           # frames per chunk (free dim of layer-1..3 matmuls)
FRT_PER_CH = CH // P
KS = HID // P      # 8 k-subtiles
F32 = mybir.dt.float32
F32R = mybir.dt.float32r

# misc constant tile column layout ([128, 256] f32, host-packed)
MC_B2 = 0          # cols 0..7   : b2 striped [128, 8]
MC_B3 = 8          # cols 8..15  : b3 striped
MC_B5 = 17         # col 17      : b5 replicated down partitions
MC_ID = 128        # cols 128..159, rows 0..31: 32x32 identity
# f32r matmul-constants tile ([128, 128])
MM_ONES = 0        # cols 0..7   : ones columns (denom matmul rhs, N=8)
MM_W7 = 16         # cols 16..95 : W7 as [128, 8, 10]
# row constants tile ([1, 192] f32r, host-packed)
RW_ONES = 0        # cols 0..127 : ones row
RW_B7 = 128        # cols 128..137 : b7


def _segment_ids(lengths: np.ndarray, total: int) -> np.ndarray:
    """Replicate jnp.repeat(arange(n), lengths, total_repeat_length=total)."""
    lengths = np.asarray(lengths, dtype=np.int64)
    seg = np.repeat(np.arange(lengths.shape[0], dtype=np.int32), np.maximum(lengths, 0))
    if seg.shape[0] >= total:
        return seg[:total]
    pad_val = seg[-1] if seg.shape[0] > 0 else np.int32(0)
    return np.concatenate([seg, np.full(total - seg.shape[0], pad_val, np.int32)])


def _balance_segments(lengths: np.ndarray) -> list[list[int]]:
    """Assign 256 segments to 8 cores, 32 each, minimizing max frame count."""
    order = np.argsort(-lengths, kind="stable")
    loads = [0] * NCORES
    bins: list[list[int]] = [[] for _ in range(NCORES)]
    for s in order:
        cands = [c for c in range(NCORES) if len(bins[c]) < SEGS_PER_CORE]
        c = min(cands, key=lambda c: (loads[c], c))
        bins[c].append(int(s))
        loads[c] += int(lengths[s])
    for b in bins:
        b.sort()
    return bins


UNROLL = 4         # chunks per hardware-loop iteration


def _build_program(m_pad: int):
    """Emit the Bass/Tile program for one core with m_pad frames (static).

    Chunks 0 and nch-1 are peeled (they carry the PSUM accumulation
    start/stop flags); the middle chunks run in a Tile hardware loop
    (For_i) so per-engine semaphore counts reset every back-edge and the
    instruction stream stays small.
    """
    nch = m_pad // CH
    frt = m_pad // P
    S = SEGS_PER_CORE

    nc = bacc.Bacc("TRN2", target_bir_lowering=False, debug=False,
                   num_devices=NCORES)

    xT_d = nc.dram_tensor("xT", [P, m_pad], F32R, kind="ExternalInput")
    A_d = nc.dram_tensor("Amat", [P, frt, S], F32, kind="ExternalInput")
    W1_d = nc.dram_tensor("W1p", [P, HID], F32R, kind="ExternalInput")
    W2_d = nc.dram_tensor("W2", [HID, HID], F32R, kind="ExternalInput")
    W3_d = nc.dram_tensor("W3", [HID, HID], F32R, kind="ExternalInput")
    W4_d = nc.dram_tensor("W4", [HID, HID], F32R, kind="ExternalInput")
    W5_d = nc.dram_tensor("W5rep", [P, HID], F32, kind="ExternalInput")
    W6_d = nc.dram_tensor("W6", [HID, HID], F32R, kind="ExternalInput")
    b4_d = nc.dram_tensor("b4r", [1, HID], F32R, kind="ExternalInput")
    b6_d = nc.dram_tensor("b6r", [1, HID], F32R, kind="ExternalInput")
    misc_d = nc.dram_tensor("miscc", [P, 256], F32, kind="ExternalInput")
    mmc_d = nc.dram_tensor("mmcc", [P, P], F32R, kind="ExternalInput")
    row_d = nc.dram_tensor("rowm", [1, 192], F32R, kind="ExternalInput")
    out_d = nc.dram_tensor("out", [S, NCLS], F32, kind="ExternalOutput")

    RELU = mybir.ActivationFunctionType.Relu
    EXP = mybir.ActivationFunctionType.Exp
    MULT = mybir.AluOpType.mult
    ADD = mybir.AluOpType.add

    with tile.TileContext(nc) as tc:
        with (
            tc.tile_pool(name="wpool", bufs=1) as wpool,
            tc.tile_pool(name="xpool", bufs=2) as xpool,
            tc.tile_pool(name="apool", bufs=2) as apool,
            tc.tile_pool(name="hpool", bufs=1) as hpool,
            tc.tile_pool(name="h4pool", bufs=2) as h4pool,
            tc.tile_pool(name="spool", bufs=1) as spool,
            tc.tile_pool(name="colpool", bufs=2) as colpool,
            tc.tile_pool(name="epool", bufs=2) as epool,
            tc.tile_pool(name="psA", bufs=3, space="PSUM") as psA,
            tc.tile_pool(name="psB", bufs=2, space="PSUM") as psB,
            tc.tile_pool(name="psAcc", bufs=1, space="PSUM") as psAcc,
        ):
            # ---- resident constants/weights ----
            W1s = wpool.tile([P, HID], F32R, tag="W1")
            nc.sync.dma_start(W1s[:], W1_d.ap())
            # per-k-subtile weight tiles: chunk-0 matmuls only wait on the
            # 0.5MB slice they read, not the whole 4MB matrix
            def load_wk(d, tagp):
                tiles = []
                for k in range(KS):
                    t = wpool.tile([P, HID], F32R, tag=f"{tagp}{k}")
                    nc.sync.dma_start(t[:], d.ap()[k * P:(k + 1) * P, :])
                    tiles.append(t)
                return tiles
            W2s = load_wk(W2_d, "W2k")
            W3s = load_wk(W3_d, "W3k")
            W4s = load_wk(W4_d, "W4k")
            W5s = wpool.tile([P, HID], F32, tag="W5")
            nc.sync.dma_start(W5s[:], W5_d.ap())
            b4s = wpool.tile([1, HID], F32R, tag="b4")
            nc.sync.dma_start(b4s[:], b4_d.ap())
            b6s = wpool.tile([1, HID], F32R, tag="b6")
            nc.sync.dma_start(b6s[:], b6_d.ap())
            misc = wpool.tile([P, 256], F32, tag="misc")
            nc.sync.dma_start(misc[:], misc_d.ap())
            mmc = wpool.tile([P, P], F32R, tag="mmc")
            nc.sync.dma_start(mmc[:], mmc_d.ap())
            rowm = wpool.tile([1, 192], F32R, tag="rowm")
            nc.sync.dma_start(rowm[:], row_d.ap())

            ones_row = rowm[:, RW_ONES:RW_ONES + P]
            ones_col = mmc[:, MM_ONES:MM_ONES + 8]
            b5col = misc[:, MC_B5:MC_B5 + 1]
            ident = misc[:S, MC_ID:MC_ID + S]
            W7v = mmc[:, MM_W7:MM_W7 + KS * NCLS].rearrange(
                "p (o c) -> p o c", c=NCLS)
            b7row = rowm[:, RW_B7:RW_B7 + NCLS]

            # persistent PSUM accumulators (own banks for the whole pass)
            pooled0 = psAcc.tile([S, 512], F32, tag="pooled0")
            pooled1 = psAcc.tile([S, 512], F32, tag="pooled1")
            denom = psAcc.tile([S, 8], F32, tag="denom")

            # ---- main pass over frame chunks ----
            def chunk_group(c0, n_chunks, first=False, last=False):
                """Emit n_chunks chunks starting at chunk index c0 (int or
                loop ScalarValue). first/last carry PSUM group flags."""
                xg = xpool.tile([P, UNROLL * CH], F32R, tag="x")
                nc.sync.dma_start(
                    xg[:, :n_chunks * CH],
                    xT_d.ap()[:, bass.ds(c0 * CH, n_chunks * CH)])
                ag = apool.tile([P, UNROLL * FRT_PER_CH, S], F32, tag="A")
                nc.sync.dma_start(
                    ag[:, :n_chunks * FRT_PER_CH, :],
                    A_d.ap()[:, bass.ds(c0 * FRT_PER_CH,
                                        n_chunks * FRT_PER_CH), :])

                for u in range(n_chunks):
                    xt = xg[:, u * CH:(u + 1) * CH]
                    # L1 (b1 folded via ones feature)
                    h1 = hpool.tile([P, KS, CH], F32R, tag="hA")
                    for m in range(KS):
                        ps = psA.tile([P, CH], F32, tag="mm")
                        nc.tensor.matmul(ps[:], W1s[:, m * P:(m + 1) * P], xt,
                                         start=True, stop=True)
                        nc.scalar.activation(h1[:, m, :], ps[:], RELU)

                    # L2 / L3 (h3 reuses h1's slot; h1 dead once L2 done)
                    h_in = h1
                    for Ws, boff, tag in ((W2s, MC_B2, "hB"), (W3s, MC_B3, "hA")):
                        h_out = hpool.tile([P, KS, CH], F32R, tag=tag)
                        for m in range(KS):
                            ps = psA.tile([P, CH], F32, tag="mm")
                            for k in range(KS):
                                nc.tensor.matmul(
                                    ps[:], Ws[k][:, m * P:(m + 1) * P],
                                    h_in[:, k, :],
                                    start=(k == 0), stop=(k == KS - 1))
                            nc.scalar.activation(
                                h_out[:, m, :], ps[:], RELU,
                                bias=misc[:, boff + m:boff + m + 1])
                        h_in = h_out
                    h3 = h_in

                    # L4 (frame-major) + scores + pooling per 128-frame tile
                    for f in range(FRT_PER_CH):
                        h4 = h4pool.tile([P, HID], F32R, tag="h4")
                        for n in range(2):
                            ps4 = psB.tile([P, 512], F32, tag="l4")
                            for k in range(KS):
                                nc.tensor.matmul(
                                    ps4[:], h3[:, k, f * P:(f + 1) * P],
                                    W4s[k][:, n * 512:(n + 1) * 512],
                                    start=(k == 0), stop=False)
                            nc.tensor.matmul(ps4[:], ones_row,
                                             b4s[:, n * 512:(n + 1) * 512],
                                             start=False, stop=True)
                            nc.scalar.activation(h4[:, n * 512:(n + 1) * 512],
                                                 ps4[:], RELU)

                        # scores: d = sum(h4*W5rep); e = max(exp(d + b5), 1)
                        prod = spool.tile([P, HID], F32, tag="sc")
                        ct = colpool.tile([P, 16], F32, tag="col")
                        nc.vector.tensor_tensor(
                            out=prod[:], in0=h4.bitcast(F32)[:], in1=W5s[:],
                            op=MULT)
                        nc.vector.tensor_reduce(
                            out=ct[:, 0:1], in_=prod[:],
                            axis=mybir.AxisListType.X, op=ADD)
                        nc.scalar.activation(ct[:, 1:2], ct[:, 0:1], EXP,
                                             bias=b5col)
                        nc.vector.tensor_scalar_max(ct[:, 2:3], ct[:, 1:2], 1.0)
                        et = epool.tile([P, S], F32R, tag="E")
                        nc.vector.tensor_scalar_mul(
                            et[:], ag[:, u * FRT_PER_CH + f, :], ct[:, 2:3])

                        st = bool(first and u == 0 and f == 0)
                        sp = bool(last and u == n_chunks - 1
                                  and f == FRT_PER_CH - 1)
                        nc.tensor.matmul(pooled0[:], et[:], h4[:, :512],
                                         start=st, stop=sp)
                        nc.tensor.matmul(pooled1[:], et[:], h4[:, 512:],
                                         start=st, stop=sp)
                        nc.tensor.matmul(denom[:], et[:], ones_col,
                                         start=st, stop=sp)

            # peel chunk 0 (PSUM group start) and chunk nch-1 (stop)
            import os
            chunk_group(0, 1, first=True)
            if os.environ.get("KERNEL_STATIC_UNROLL"):
                # cost-model twin: same stream, no dynamic loop machinery
                c = 1
                while c < nch - 1:
                    n = min(UNROLL, nch - 1 - c)
                    chunk_group(c, n)
                    c += n
            elif nch > 2:
                # 8 chunks per back-edge, emitted as 4-chunk DMA groups so
                # the x/A staging tiles stay at 4*CH
                def loop_body(iv, unroll):
                    off = 0
                    while off < unroll:
                        n = min(UNROLL, unroll - off)
                        chunk_group(iv + off, n)
                        off += n
                tc.For_i_unrolled_general(
                    start=1, end=nch - 1, step=1,
                    unrollable_body=loop_body,
                    max_unroll=2 * UNROLL,
                    hint_engines=(mybir.EngineType.PE,),
                )
            chunk_group(nch - 1, 1, last=True)

            # ---- final per-utterance MLP ----
            # W6 reuses W4's SBUF slots (W4 is dead after the last chunk)
            W6s = load_wk(W6_d, "W4k")

            fc = colpool.tile([S, 16], F32, tag="col")
            nc.vector.tensor_copy(out=fc[:, 0:1], in_=denom[:, 0:1])
            nc.vector.reciprocal(fc[:, 1:2], fc[:, 0:1])

            pooled_sb = spool.tile([S, HID], F32, tag="sc")
            nc.vector.tensor_scalar_mul(pooled_sb[:, :512], pooled0[:], fc[:, 1:2])
            nc.vector.tensor_scalar_mul(pooled_sb[:, 512:], pooled1[:], fc[:, 1:2])

            # transpose pooled -> pooledT [hid, seg]
            tposed = wpool.tile([P, KS, 2 * S], F32R, tag="tposed")
            pooledT = tposed[:, :, :S]
            gT = tposed[:, :, S:]
            for k in range(KS):
                pst = psA.tile([P, S], F32, tag="mm")
                nc.tensor.transpose(pst[:], pooled_sb[:, k * P:(k + 1) * P],
                                    ident)
                nc.vector.tensor_copy(out=pooledT[:, k, :], in_=pst[:])

            # g = relu(pooled @ W6 + b6)   (seg-major [S, HID])
            g_sb = spool.tile([S, HID], F32, tag="sc")
            for n in range(2):
                psg = psB.tile([S, 512], F32, tag="l4")
                for k in range(KS):
                    nc.tensor.matmul(psg[:], pooledT[:, k, :],
                                     W6s[k][:, n * 512:(n + 1) * 512],
                                     start=(k == 0), stop=False)
                nc.tensor.matmul(psg[:], ones_row[:, :S],
                                 b6s[:, n * 512:(n + 1) * 512],
                                 start=False, stop=True)
                nc.scalar.activation(g_sb[:, n * 512:(n + 1) * 512], psg[:], RELU)

            # gT [hid, seg]
            for k in range(KS):
                pst = psA.tile([P, S], F32, tag="mm")
                nc.tensor.transpose(pst[:], g_sb[:, k * P:(k + 1) * P], ident)
                nc.vector.tensor_copy(out=gT[:, k, :], in_=pst[:])

            # out = g @ W7 + b7
            pso = psA.tile([S, NCLS], F32, tag="mm")
            for k in range(KS):
                nc.tensor.matmul(pso[:], gT[:, k, :], W7v[:, k, :],
                                 start=(k == 0), stop=False)
            nc.tensor.matmul(pso[:], ones_row[:, :S], b7row,
                             start=False, stop=True)
            oc = colpool.tile([S, 16], F32, tag="col")
            nc.vector.tensor_copy(out=oc[:, :NCLS], in_=pso[:])
            nc.sync.dma_start(out_d.ap()[:], oc[:, :NCLS])

    nc.compile()
    return nc


def prepare_inputs(x, W1, b1, W2, b2, W3, b3, W4, b4, W5, b5, W6, b6, W7, b7,
                   lengths):
    """Host-side sharding/packing. Returns (in_maps, bins, m_pad)."""
    x = np.ascontiguousarray(np.asarray(x, dtype=np.float32))
    lengths = np.asarray(lengths)
    total = x.shape[0]
    seg_ids = _segment_ids(lengths, total)
    counts = np.bincount(seg_ids, minlength=NSEG).astype(np.int64)
    starts = np.zeros(NSEG + 1, dtype=np.int64)
    starts[1:] = np.cumsum(counts)

    bins = _balance_segments(counts)
    core_frames = [int(sum(counts[s] for s in b)) for b in bins]
    m_pad = ((max(core_frames) + CH - 1) // CH) * CH
    frt = m_pad // P

    W1p = np.zeros((P, HID), dtype=np.float32)
    W1p[:FEAT] = np.asarray(W1, dtype=np.float32)
    W1p[FEAT] = np.asarray(b1, dtype=np.float32)

    misc = np.zeros((P, 256), dtype=np.float32)
    misc[:, MC_B2:MC_B2 + KS] = np.asarray(b2, np.float32).reshape(KS, P).T
    misc[:, MC_B3:MC_B3 + KS] = np.asarray(b3, np.float32).reshape(KS, P).T
    misc[:, MC_B5] = np.float32(np.asarray(b5, np.float32).reshape(-1)[0])
    misc[:SEGS_PER_CORE, MC_ID:MC_ID + SEGS_PER_CORE] = np.eye(
        SEGS_PER_CORE, dtype=np.float32)

    mmcc = np.zeros((P, P), dtype=np.float32)
    mmcc[:, MM_ONES:MM_ONES + 8] = 1.0
    mmcc[:, MM_W7:MM_W7 + KS * NCLS] = np.asarray(W7, np.float32).reshape(
        KS, P, NCLS).transpose(1, 0, 2).reshape(P, KS * NCLS)

    rowm = np.zeros((1, 192), dtype=np.float32)
    rowm[0, RW_ONES:RW_ONES + P] = 1.0
    rowm[0, RW_B7:RW_B7 + NCLS] = np.asarray(b7, np.float32).reshape(-1)

    shared = dict(
        W1p=W1p,
        W2=np.ascontiguousarray(np.asarray(W2, np.float32)),
        W3=np.ascontiguousarray(np.asarray(W3, np.float32)),
        W4=np.ascontiguousarray(np.asarray(W4, np.float32)),
        W5rep=np.broadcast_to(np.asarray(W5, np.float32).reshape(1, HID),
                              (P, HID)).copy(),
        W6=np.ascontiguousarray(np.asarray(W6, np.float32)),
        b4r=np.asarray(b4, np.float32).reshape(1, HID),
        b6r=np.asarray(b6, np.float32).reshape(1, HID),
        miscc=misc,
        mmcc=mmcc,
        rowm=rowm,
    )

    in_maps = []
    for core in range(NCORES):
        segs = bins[core]
        xs = [x[starts[s]:starts[s + 1]] for s in segs]
        xcat = np.concatenate(xs, axis=0) if xs else np.zeros((0, FEAT), np.float32)
        n = xcat.shape[0]
        xT = np.zeros((P, m_pad), dtype=np.float32)
        xT[:FEAT, :n] = xcat.T
        xT[FEAT, :n] = 1.0  # constant feature -> b1
        A = np.zeros((m_pad, SEGS_PER_CORE), dtype=np.float32)
        off = 0
        for j, s in enumerate(segs):
            ln = int(counts[s])
            A[off:off + ln, j] = 1.0
            off += ln
        im = dict(shared)
        im["xT"] = xT
        # partition-major layout [P, frt, S]: Ah[p, t, s] = A[t*128 + p, s]
        im["Amat"] = np.ascontiguousarray(
            A.reshape(frt, P, SEGS_PER_CORE).transpose(1, 0, 2))
        in_maps.append(im)
    return in_maps, bins, m_pad


_PROGRAM_CACHE: dict[int, object] = {}


def kernel(**inputs) -> np.ndarray:
    in_maps, bins, m_pad = prepare_inputs(**inputs)
    nc = _PROGRAM_CACHE.get(m_pad)
    if nc is None:
        nc = _build_program(m_pad)
        _PROGRAM_CACHE[m_pad] = nc
    res = run_bass_kernel_spmd(nc, in_maps, core_ids=list(range(NCORES)))
    out = np.zeros((NSEG, NCLS), dtype=np.float32)
    for core in range(NCORES):
        out[bins[core]] = res.results[core]["out"]
    return out



# revision 65
# speedup vs baseline: 2.5884x; 2.5884x over previous
"""Trainium2 Bass kernel for nn_Dnn_with_Attention (ragged attention-pooled DNN).

Contract: kernel(**inputs) takes FULL unsharded numpy inputs (keys as in
reference.setup_inputs()) and returns the FULL [256, 10] float32 output.

Strategy (data-parallel over utterances, 8 NeuronCores):
  - Host: greedily balance the 256 segments over 8 cores (32 whole segments
    each), gather each core's frames, transpose x to feature-major
    [128(feat-padded), M_PAD] and build a per-frame one-hot segment
    membership matrix A [M_PAD/128, 128, 32].  A row of ones is appended as
    feature 78 so b1 folds into W1.
  - Device (per core): L1 runs in float32r; the three HID x HID layers
    (L2/L3/L4) run as fp8(e4m3) DoubleRow matmuls (two 128-row contraction
    tiles per pass, 0.5 cycles/row on the PE) — weights and the
    inter-layer activations are quantized to e4m3, which keeps the final
    relative error ~1e-2 (measured against the fp64 reference on the
    actual problem inputs).  L2/L3 activations (relu+bias+fp8 convert) run
    on the scalar engine, L1 relu on DVE, L4 relu on gpsimd; the attention
    score dot-product is a single fused DVE tensor_tensor_reduce.  b4 is
    added via a tiny fp8 ones-row DoubleRow matmul into the same PSUM
    accumulation group.
  - Segment softmax pooling as small float32r PE matmuls E.T @ h4
    (E = A * e) accumulated into persistent PSUM across all chunks; the
    softmax denominator comes from E.T @ ones.  The final per-utterance
    MLP runs once at the end in float32r.
"""

import sys

sys.path.insert(0, "/opt/trn_rl_repo")

import numpy as np
import ml_dtypes

import concourse.bass as bass
import concourse.mybir as mybir
import concourse.tile as tile
from concourse import bacc
from concourse.bass_utils import run_bass_kernel_spmd

P = 128
FEAT = 78
HID = 1024
NCLS = 10
NSEG = 256
NCORES = 8
SEGS_PER_CORE = NSEG // NCORES
CH = 512           # frames per chunk (free dim of layer-1..3 matmuls)
FRT_PER_CH = CH // P
NCOL = 256         # moving-dim columns per DoubleRow matmul (HW limit)
KS = HID // P      # 8 k-subtiles
KP = KS // 2       # 4 DoubleRow k-pairs
F32 = mybir.dt.float32
F32R = mybir.dt.float32r
F8 = mybir.dt.float8e4
BF16 = mybir.dt.bfloat16
E4NP = ml_dtypes.float8_e4m3

# misc constant tile column layout ([128, 256] f32, host-packed)
MC_B2 = 0          # cols 0..7   : b2 striped [128, 8]
MC_B3 = 8          # cols 8..15  : b3 striped
MC_B5 = 17         # col 17      : b5 replicated down partitions
MC_ID = 128        # cols 128..159, rows 0..31: 32x32 identity
MC_ID2 = 160       # cols 160..191, rows 32..63: 32x32 identity (2nd block)
# f32r matmul-constants tile ([128, 128])
MM_ONES = 0        # cols 0..7   : ones columns (denom matmul rhs, N=8)
MM_W7 = 16         # cols 16..95 : W7 as [128, 8, 10]
# row constants tile ([1, 192] f32r, host-packed)
RW_ONES = 0        # cols 0..127 : ones row
RW_B7 = 128        # cols 128..137 : b7


def _segment_ids(lengths: np.ndarray, total: int) -> np.ndarray:
    """Replicate jnp.repeat(arange(n), lengths, total_repeat_length=total)."""
    lengths = np.asarray(lengths, dtype=np.int64)
    seg = np.repeat(np.arange(lengths.shape[0], dtype=np.int32), np.maximum(lengths, 0))
    if seg.shape[0] >= total:
        return seg[:total]
    pad_val = seg[-1] if seg.shape[0] > 0 else np.int32(0)
    return np.concatenate([seg, np.full(total - seg.shape[0], pad_val, np.int32)])


def _balance_segments(lengths: np.ndarray) -> list[list[int]]:
    """Assign 256 segments to 8 cores, 32 each, minimizing max frame count."""
    order = np.argsort(-lengths, kind="stable")
    loads = [0] * NCORES
    bins: list[list[int]] = [[] for _ in range(NCORES)]
    for s in order:
        cands = [c for c in range(NCORES) if len(bins[c]) < SEGS_PER_CORE]
        c = min(cands, key=lambda c: (loads[c], c))
        bins[c].append(int(s))
        loads[c] += int(lengths[s])
    for b in bins:
        b.sort()
    return bins


UNROLL = 8         # chunks per hardware-loop iteration


def _build_program(m_pad: int):
    """Emit the Bass/Tile program for one core with m_pad frames (static).

    Chunks 0 and nch-1 are peeled (they carry the PSUM accumulation
    start/stop flags); the middle chunks run in a Tile hardware loop
    (For_i) so per-engine semaphore counts reset every back-edge and the
    instruction stream stays small.
    """
    nch = m_pad // CH
    frt = m_pad // P
    S = SEGS_PER_CORE

    nc = bacc.Bacc("TRN2", target_bir_lowering=False, debug=False,
                   num_devices=NCORES)

    xT_d = nc.dram_tensor("xq", [P // 2, 2, m_pad], F8, kind="ExternalInput")
    A_d = nc.dram_tensor("Amat", [P, frt, S], F32, kind="ExternalInput")
    W1_d = nc.dram_tensor("W1q", [P // 2, 2, HID], F8, kind="ExternalInput")
    W2_d = nc.dram_tensor("W2q", [KP, P, 2, HID], F8, kind="ExternalInput")
    W3_d = nc.dram_tensor("W3q", [KP, P, 2, HID], F8, kind="ExternalInput")
    W4_d = nc.dram_tensor("W4q", [KP, P, 2, HID], F8, kind="ExternalInput")
    W5_d = nc.dram_tensor("W5rep", [P, HID], BF16, kind="ExternalInput")
    W6_d = nc.dram_tensor("W6", [HID, HID], F32R, kind="ExternalInput")
    b4_d = nc.dram_tensor("b4q", [1, 2, HID], F8, kind="ExternalInput")
    on_d = nc.dram_tensor("onesq", [1, 2, P], F8, kind="ExternalInput")
    b6_d = nc.dram_tensor("b6r", [1, HID], F32R, kind="ExternalInput")
    misc_d = nc.dram_tensor("miscc", [P, 256], F32, kind="ExternalInput")
    mmc_d = nc.dram_tensor("mmcc", [P, P], F32R, kind="ExternalInput")
    row_d = nc.dram_tensor("rowm", [1, 192], F32R, kind="ExternalInput")
    out_d = nc.dram_tensor("out", [S, NCLS], F32, kind="ExternalOutput")

    RELU = mybir.ActivationFunctionType.Relu
    EXP = mybir.ActivationFunctionType.Exp
    MULT = mybir.AluOpType.mult
    ADD = mybir.AluOpType.add
    MAX = mybir.AluOpType.max
    DR = mybir.MatmulPerfMode.DoubleRow

    with tile.TileContext(nc) as tc:
        with (
            tc.tile_pool(name="wpool", bufs=1) as wpool,
            tc.tile_pool(name="xpool", bufs=2) as xpool,
            tc.tile_pool(name="apool", bufs=2) as apool,
            tc.tile_pool(name="hpool", bufs=2) as hpool,
            tc.tile_pool(name="h4pool", bufs=12) as h4pool,
            tc.tile_pool(name="spool", bufs=4) as spool,
            tc.tile_pool(name="colpool", bufs=8) as colpool,
            tc.tile_pool(name="epool", bufs=8) as epool,
            tc.tile_pool(name="psA", bufs=4, space="PSUM") as psA,
            tc.tile_pool(name="psB", bufs=2, space="PSUM") as psB,
            tc.tile_pool(name="psAcc", bufs=1, space="PSUM") as psAcc,
        ):
            # ---- resident constants/weights ----
            # Small tensors needed by chunk 0's L1 go on the DMA queue
            # first; the 3MB of fp8 layer weights are issued AFTER chunk 0's
            # x/A DMA (via the preload callback) so L1 can start within ~2us
            # and L2/L3/L4 weights stream in behind it.
            W1s = wpool.tile([P // 2, 2, HID], F8, tag="W1")
            nc.sync.dma_start(W1s[:], W1_d.ap())
            misc = wpool.tile([P, 256], F32, tag="misc")
            nc.sync.dma_start(misc[:], misc_d.ap())
            mmc = wpool.tile([P, P], F32R, tag="mmc")
            nc.sync.dma_start(mmc[:], mmc_d.ap())
            rowm = wpool.tile([1, 192], F32R, tag="rowm")
            nc.sync.dma_start(rowm[:], row_d.ap())
            b4qs = wpool.tile([1, 2, HID], F8, tag="b4q")
            nc.sync.dma_start(b4qs[:], b4_d.ap())
            onesq = wpool.tile([1, 2, P], F8, tag="onesq")
            nc.sync.dma_start(onesq[:], on_d.ap())
            b6s = wpool.tile([1, HID], F32R, tag="b6")
            nc.sync.dma_start(b6s[:], b6_d.ap())

            W2q = [wpool.tile([P, 2, HID], F8, tag=f"W2q{j}", name=f"W2q{j}")
                   for j in range(KP)]
            W3q = [wpool.tile([P, 2, HID], F8, tag=f"W3q{j}", name=f"W3q{j}")
                   for j in range(KP)]
            W4q = [wpool.tile([P, 2, HID], F8, tag=f"W4q{j}", name=f"W4q{j}")
                   for j in range(KP)]
            W5s = wpool.tile([P, HID], BF16, tag="W5")

            def load_big_weights():
                for d, tiles in ((W2_d, W2q), (W3_d, W3q), (W4_d, W4q)):
                    for j in range(KP):
                        nc.sync.dma_start(tiles[j][:], d.ap()[j])
                nc.sync.dma_start(W5s[:], W5_d.ap())

            ones_row = rowm[:, RW_ONES:RW_ONES + P]
            ones_col = mmc[:, MM_ONES:MM_ONES + 8]
            b5col = misc[:, MC_B5:MC_B5 + 1]
            ident = misc[:S, MC_ID:MC_ID + S]
            W7v = mmc[:, MM_W7:MM_W7 + KS * NCLS].rearrange(
                "p (o c) -> p o c", c=NCLS)
            b7row = rowm[:, RW_B7:RW_B7 + NCLS]

            # persistent PSUM accumulators (own banks for the whole pass);
            # the softmax denominator is accumulated in SBUF by the DVE
            # (eacc += et) and partition-reduced once in the tail, which
            # frees a PSUM bank for a 4th psA buffer
            pooled0 = psAcc.tile([S, 512], F32, tag="pooled0")
            pooled1 = psAcc.tile([S, 512], F32, tag="pooled1")
            # eacc must be f32r-typed (it feeds an f32r matmul in the tail;
            # walrus requires producers of f32r-matmul operands to round);
            # instead of a memset, the very first accumulation is a copy
            eacc = wpool.tile([P, S], F32R, tag="eacc")

            # ---- main pass over frame chunks ----
            # Pooling matmuls are deferred by one chunk (emitted after the
            # NEXT chunk's L3) so the PE never stalls on the cross-engine
            # score chain (gpsimd relu -> DVE reduce -> scalar exp -> DVE
            # mul) that produces et.
            pend: list[tuple] = []

            def flush_pool():
                while pend:
                    et, h4, st, sp = pend.pop(0)
                    nc.tensor.matmul(pooled0[:], et[:], h4[:, :512],
                                     start=st, stop=sp)
                    nc.tensor.matmul(pooled1[:], et[:], h4[:, 512:],
                                     start=st, stop=sp)

            def chunk_group(c0, n_chunks, first=False, last=False,
                            preload=None):
                """Emit n_chunks chunks starting at chunk index c0 (int or
                loop ScalarValue). first/last carry PSUM group flags."""
                xg = xpool.tile([P // 2, 2, UNROLL * CH], F8, tag="x")
                nc.sync.dma_start(
                    xg[:, :, :n_chunks * CH],
                    xT_d.ap()[:, :, bass.ds(c0 * CH, n_chunks * CH)])
                ag = apool.tile([P, UNROLL * FRT_PER_CH, S], F32, tag="A")
                nc.sync.dma_start(
                    ag[:, :n_chunks * FRT_PER_CH, :],
                    A_d.ap()[:, bass.ds(c0 * FRT_PER_CH,
                                        n_chunks * FRT_PER_CH), :])
                if preload is not None:
                    preload()

                # --- per-chunk layer emitters (skewed pipeline below) ---
                h1s: dict[int, object] = {}
                h2s: dict[int, object] = {}
                h3s: dict[int, object] = {}

                def do_L1(u):
                    # fp8 DoubleRow over two 64-feature halves (b1 folded
                    # via ones feature); relu on DVE + gpsimd
                    h1 = hpool.tile([P, KS, CH], F8, tag="h1")
                    for m in range(KS):
                        ps = psA.tile([P, CH], F32, tag="mm")
                        for c2 in range(CH // NCOL):
                            xt = xg[:, :, u * CH + c2 * NCOL:
                                    u * CH + (c2 + 1) * NCOL]
                            nc.tensor.matmul(ps[:, c2 * NCOL:(c2 + 1) * NCOL],
                                             W1s[:, :, m * P:(m + 1) * P],
                                             xt, start=True, stop=True,
                                             perf_mode=DR)
                        nc.vector.tensor_scalar_max(h1[:, m, :], ps[:], 0.0)
                    h1s[u] = h1

                def do_L23(u, Wq, boff, li):
                    # fp8 DoubleRow; relu+bias on scalar (L3 m<2 on DVE)
                    h_in = h1s.pop(u) if li == 2 else h2s.pop(u)
                    h_out = hpool.tile([P, KS, CH], F8,
                                       tag=("h2" if li == 2 else "h3"))
                    for m in range(KS):
                        ps = psA.tile([P, CH], F32, tag="mm")
                        for c2 in range(CH // NCOL):
                            seg = ps[:, c2 * NCOL:(c2 + 1) * NCOL]
                            for j in range(KP):
                                nc.tensor.matmul(
                                    seg, Wq[j][:, :, m * P:(m + 1) * P],
                                    h_in[:, 2 * j:2 * j + 2,
                                         c2 * NCOL:(c2 + 1) * NCOL],
                                    start=(j == 0), stop=(j == KP - 1),
                                    perf_mode=DR)
                        bcol = misc[:, boff + m:boff + m + 1]
                        nc.scalar.activation(
                            h_out[:, m, :], ps[:], RELU, bias=bcol)
                    (h2s if li == 2 else h3s)[u] = h_out

                h4s: dict[int, list] = {}

                def do_L4(u):
                    # frame-major fp8 DR; relu on DVE (n=0) / scalar (n=1);
                    # h4 is bf16 so the score reduction runs at DVE 2x rate
                    # and pooling matmuls stay 1 cycle/row
                    h3 = h3s.pop(u)
                    tiles = []
                    for f in range(FRT_PER_CH):
                        h4 = h4pool.tile([P, HID], BF16, tag="h4")
                        for n in range(2):
                            ps4 = psB.tile([P, 512], F32, tag="l4")
                            for c2 in range(2):
                                seg = ps4[:, c2 * 256:(c2 + 1) * 256]
                                col0 = n * 512 + c2 * 256
                                for j in range(KP):
                                    nc.tensor.matmul(
                                        seg,
                                        h3[:, 2 * j:2 * j + 2, f * P:(f + 1) * P],
                                        W4q[j][:, :, col0:col0 + 256],
                                        start=(j == 0), stop=False,
                                        perf_mode=DR)
                                nc.tensor.matmul(
                                    seg, onesq[:],
                                    b4qs[:, :, col0:col0 + 256],
                                    start=False, stop=True, perf_mode=DR)
                            if n == 0:
                                nc.vector.tensor_scalar_max(
                                    h4[:, :512], ps4[:], 0.0)
                            else:
                                nc.scalar.activation(h4[:, 512:], ps4[:],
                                                     RELU)
                        tiles.append(h4)
                    h4s[u] = tiles

                def do_scores(u):
                    # d = sum(h4*W5rep); e = max(exp(d + b5), 1); E = A*e.
                    # tensor_tensor_reduce crashes this HW build, so the
                    # reduction runs as gpsimd multiply (the Pool engine is
                    # otherwise idle) + DVE tensor_scalar with accum_out.
                    for f, h4 in enumerate(h4s.pop(u)):
                        prod = spool.tile([P, HID], BF16, tag="sc")
                        ct = colpool.tile([P, 16], F32, tag="col")
                        nc.gpsimd.tensor_tensor(
                            out=prod[:], in0=h4[:], in1=W5s[:], op=MULT)
                        nc.vector.tensor_scalar(
                            out=prod[:], in0=prod[:], scalar1=1.0,
                            scalar2=0.0, op0=MULT, op1=ADD,
                            accum_out=ct[:, 0:1])
                        nc.scalar.activation(ct[:, 1:2], ct[:, 0:1], EXP,
                                             bias=b5col)
                        nc.gpsimd.tensor_scalar_max(ct[:, 2:3], ct[:, 1:2], 1.0)
                        et = epool.tile([P, S], BF16, tag="E")
                        nc.gpsimd.tensor_scalar_mul(
                            et[:], ag[:, u * FRT_PER_CH + f, :], ct[:, 2:3])
                        if first and u == 0 and f == 0:
                            nc.vector.tensor_copy(out=eacc[:], in_=et[:])
                        else:
                            nc.vector.tensor_tensor(
                                out=eacc[:], in0=eacc.bitcast(F32)[:],
                                in1=et[:], op=ADD)

                        st = bool(first and u == 0 and f == 0)
                        sp = bool(last and u == n_chunks - 1
                                  and f == FRT_PER_CH - 1)
                        pend.append((et, h4, st, sp))

                # --- 4-deep skewed pipeline over this group's chunks: at
                # step s the PE runs L2(s-1), L3(s-2), L1(s), L4(s-3), so
                # every matmul's inputs were activated a full step (~13us
                # of PE work) earlier and no act->matmul chain is on the
                # critical path.  Layer order within the step is chosen so
                # each layer's psA buffers were last touched by acts that
                # are already 1+ layers old.  The score chain trails L4 by
                # a step and pooling trails scores by one more.
                for s in range(n_chunks + 5):
                    if 0 <= s - 1 < n_chunks:
                        do_L23(s - 1, W2q, MC_B2, 2)
                    if 0 <= s - 2 < n_chunks:
                        do_L23(s - 2, W3q, MC_B3, 3)
                    if s < n_chunks:
                        do_L1(s)
                    flush_pool()
                    if 0 <= s - 3 < n_chunks:
                        do_L4(s - 3)
                    if 0 <= s - 4 < n_chunks:
                        do_scores(s - 4)

                # pend MUST be empty at every chunk_group return: entries may
                # not cross a For_i trace boundary (buffer rotation would
                # resolve to the wrong copies at runtime, and chunk 0's
                # start=True matmul must stay outside the loop body)
                flush_pool()

            # peel chunk 0 (PSUM group start) and chunk nch-1 (stop)
            import os
            chunk_group(0, 1, first=True, last=(nch == 1),
                        preload=load_big_weights)
            if nch == 1:
                pass
            elif os.environ.get("KERNEL_STATIC_UNROLL"):
                # cost-model twin: same stream, no dynamic loop machinery
                c = 1
                while c < nch - 1:
                    n = min(UNROLL, nch - 1 - c)
                    chunk_group(c, n)
                    c += n
            elif nch > 2:
                # 8 chunks per back-edge
                def loop_body(iv, unroll):
                    off = 0
                    while off < unroll:
                        n = min(UNROLL, unroll - off)
                        chunk_group(iv + off, n)
                        off += n
                tc.For_i_unrolled_general(
                    start=1, end=nch - 1, step=1,
                    unrollable_body=loop_body,
                    max_unroll=UNROLL,
                    hint_engines=(mybir.EngineType.PE,),
                )
            if nch > 1:
                chunk_group(nch - 1, 1, last=True)

            # W6 load: emitted here but the DMA engine runs well ahead of
            # the PE (its issue is only throttled by the x/A prefetch
            # queue), so the 4MB is resident long before the tail needs it
            W6s = []
            for k in range(KS):
                t = wpool.tile([P, HID], F32R, tag=f"W6k{k}")
                nc.sync.dma_start(t[:], W6_d.ap()[k * P:(k + 1) * P, :])
                W6s.append(t)

            # ---- final per-utterance MLP (f32r) ----
            # denom[seg] = sum_p eacc[p, seg] via one tiny PE matmul
            psd = psA.tile([S, 8], F32, tag="mm")
            nc.tensor.matmul(psd[:], eacc[:], ones_col,
                             start=True, stop=True)
            fc = colpool.tile([P, 16], F32, tag="col")
            nc.vector.tensor_copy(out=fc[:S, 0:1], in_=psd[:, 0:1])
            nc.vector.reciprocal(fc[:S, 1:2], fc[:S, 0:1])

            pooled_sb = spool.tile([S, HID], F32, tag="sc")
            nc.vector.tensor_scalar_mul(pooled_sb[:, :512], pooled0[:],
                                        fc[:S, 1:2])
            nc.vector.tensor_scalar_mul(pooled_sb[:, 512:], pooled1[:],
                                        fc[:S, 1:2])

            # transpose pooled -> pooledT [hid, seg]
            tposed = wpool.tile([P, KS, 2 * S], F32R, tag="tposed")
            pooledT = tposed[:, :, :S]
            gT = tposed[:, :, S:]
            for k in range(KS):
                pst = psA.tile([P, S], F32, tag="mm")
                nc.tensor.transpose(pst[:], pooled_sb[:, k * P:(k + 1) * P],
                                    ident)
                nc.vector.tensor_copy(out=pooledT[:, k, :], in_=pst[:])

            # g = relu(pooled @ W6 + b6)   (seg-major [S, HID])
            g_sb = spool.tile([S, HID], F32, tag="sc")
            for n in range(2):
                psg = psB.tile([S, 512], F32, tag="l4")
                for k in range(KS):
                    nc.tensor.matmul(psg[:], pooledT[:, k, :],
                                     W6s[k][:, n * 512:(n + 1) * 512],
                                     start=(k == 0), stop=False)
                nc.tensor.matmul(psg[:], ones_row[:, :S],
                                 b6s[:, n * 512:(n + 1) * 512],
                                 start=False, stop=True)
                nc.scalar.activation(g_sb[:, n * 512:(n + 1) * 512], psg[:], RELU)

            # gT [hid, seg]
            for k in range(KS):
                pst = psA.tile([P, S], F32, tag="mm")
                nc.tensor.transpose(pst[:], g_sb[:, k * P:(k + 1) * P], ident)
                nc.vector.tensor_copy(out=gT[:, k, :], in_=pst[:])

            # out = g @ W7 + b7
            pso = psA.tile([S, NCLS], F32, tag="mm")
            for k in range(KS):
                nc.tensor.matmul(pso[:], gT[:, k, :], W7v[:, k, :],
                                 start=(k == 0), stop=False)
            nc.tensor.matmul(pso[:], ones_row[:, :S], b7row,
                             start=False, stop=True)
            oc = colpool.tile([S, 16], F32, tag="col")
            nc.vector.tensor_copy(out=oc[:, :NCLS], in_=pso[:])
            nc.sync.dma_start(out_d.ap()[:], oc[:, :NCLS])

    nc.compile()
    return nc


def _q8(a: np.ndarray) -> np.ndarray:
    return np.asarray(a, dtype=np.float32).astype(E4NP)


def _pack_dr(W: np.ndarray) -> np.ndarray:
    """[1024, N] weight matrix -> DoubleRow fp8 layout [KP, 128, 2, N]."""
    return np.ascontiguousarray(
        _q8(W).reshape(KP, 2, P, -1).transpose(0, 2, 1, 3))


def prepare_inputs(x, W1, b1, W2, b2, W3, b3, W4, b4, W5, b5, W6, b6, W7, b7,
                   lengths):
    """Host-side sharding/packing. Returns (in_maps, bins, m_pad)."""
    x = np.ascontiguousarray(np.asarray(x, dtype=np.float32))
    lengths = np.asarray(lengths)
    total = x.shape[0]
    seg_ids = _segment_ids(lengths, total)
    counts = np.bincount(seg_ids, minlength=NSEG).astype(np.int64)
    starts = np.zeros(NSEG + 1, dtype=np.int64)
    starts[1:] = np.cumsum(counts)

    bins = _balance_segments(counts)
    core_frames = [int(sum(counts[s] for s in b)) for b in bins]
    m_pad = ((max(core_frames) + CH - 1) // CH) * CH
    frt = m_pad // P

    W1p = np.zeros((P, HID), dtype=np.float32)
    W1p[:FEAT] = np.asarray(W1, dtype=np.float32)
    W1p[FEAT] = np.asarray(b1, dtype=np.float32)
    # DoubleRow over two 64-feature halves: [64, 2, HID]
    W1q = np.ascontiguousarray(
        _q8(W1p).reshape(2, P // 2, HID).transpose(1, 0, 2))

    misc = np.zeros((P, 256), dtype=np.float32)
    misc[:, MC_B2:MC_B2 + KS] = np.asarray(b2, np.float32).reshape(KS, P).T
    misc[:, MC_B3:MC_B3 + KS] = np.asarray(b3, np.float32).reshape(KS, P).T
    misc[:, MC_B5] = np.float32(np.asarray(b5, np.float32).reshape(-1)[0])
    misc[:SEGS_PER_CORE, MC_ID:MC_ID + SEGS_PER_CORE] = np.eye(
        SEGS_PER_CORE, dtype=np.float32)
    misc[SEGS_PER_CORE:2 * SEGS_PER_CORE, MC_ID2:MC_ID2 + SEGS_PER_CORE] = \
        np.eye(SEGS_PER_CORE, dtype=np.float32)

    mmcc = np.zeros((P, P), dtype=np.float32)
    mmcc[:, MM_ONES:MM_ONES + 8] = 1.0
    mmcc[:, MM_W7:MM_W7 + KS * NCLS] = np.asarray(W7, np.float32).reshape(
        KS, P, NCLS).transpose(1, 0, 2).reshape(P, KS * NCLS)

    rowm = np.zeros((1, 192), dtype=np.float32)
    rowm[0, RW_ONES:RW_ONES + P] = 1.0
    rowm[0, RW_B7:RW_B7 + NCLS] = np.asarray(b7, np.float32).reshape(-1)

    b4q = np.zeros((1, 2, HID), dtype=E4NP)
    b4q[0, 0, :] = _q8(np.asarray(b4, np.float32).reshape(-1))
    onesq = np.zeros((1, 2, P), dtype=E4NP)
    onesq[0, 0, :] = np.float32(1.0)

    shared = dict(
        W1q=W1q,
        W2q=_pack_dr(np.asarray(W2, np.float32)),
        W3q=_pack_dr(np.asarray(W3, np.float32)),
        W4q=_pack_dr(np.asarray(W4, np.float32)),
        W5rep=np.broadcast_to(
            np.asarray(W5, np.float32).reshape(1, HID).astype(
                ml_dtypes.bfloat16), (P, HID)).copy(),
        W6=np.ascontiguousarray(np.asarray(W6, np.float32)),
        b4q=b4q,
        onesq=onesq,
        b6r=np.asarray(b6, np.float32).reshape(1, HID),
        miscc=misc,
        mmcc=mmcc,
        rowm=rowm,
    )

    in_maps = []
    for core in range(NCORES):
        segs = bins[core]
        xs = [x[starts[s]:starts[s + 1]] for s in segs]
        xcat = np.concatenate(xs, axis=0) if xs else np.zeros((0, FEAT), np.float32)
        n = xcat.shape[0]
        xT = np.zeros((P, m_pad), dtype=np.float32)
        xT[:FEAT, :n] = xcat.T
        xT[FEAT, :n] = 1.0  # constant feature -> b1
        A = np.zeros((m_pad, SEGS_PER_CORE), dtype=np.float32)
        off = 0
        for j, s in enumerate(segs):
            ln = int(counts[s])
            A[off:off + ln, j] = 1.0
            off += ln
        im = dict(shared)
        # fp8 x, DoubleRow halves: xq[p, i, col] = xpad[i*64 + p, col]
        im["xq"] = np.ascontiguousarray(
            _q8(xT).reshape(2, P // 2, m_pad).transpose(1, 0, 2))
        # partition-major layout [P, frt, S]: Ah[p, t, s] = A[t*128 + p, s]
        im["Amat"] = np.ascontiguousarray(
            A.reshape(frt, P, SEGS_PER_CORE).transpose(1, 0, 2))
        in_maps.append(im)
    return in_maps, bins, m_pad


_PROGRAM_CACHE: dict[int, object] = {}


def kernel(**inputs) -> np.ndarray:
    in_maps, bins, m_pad = prepare_inputs(**inputs)
    nc = _PROGRAM_CACHE.get(m_pad)
    if nc is None:
        nc = _build_program(m_pad)
        _PROGRAM_CACHE[m_pad] = nc
    res = run_bass_kernel_spmd(nc, in_maps, core_ids=list(range(NCORES)))
    out = np.zeros((NSEG, NCLS), dtype=np.float32)
    for core in range(NCORES):
        out[bins[core]] = res.results[core]["out"]
    return out


# revision 76
# speedup vs baseline: 3.2102x; 1.2402x over previous
"""Trainium2 Bass kernel for nn_Dnn_with_Attention (ragged attention-pooled DNN).

Contract: kernel(**inputs) takes FULL unsharded numpy inputs (keys as in
reference.setup_inputs()) and returns the FULL [256, 10] float32 output.

Strategy (data-parallel over utterances, 8 NeuronCores):
  - Host: greedily balance the 256 segments over 8 cores (32 whole segments
    each), gather each core's frames, transpose x to feature-major
    [128(feat-padded), M_PAD] and build a per-frame one-hot segment
    membership matrix A [M_PAD/128, 128, 32].  A row of ones is appended as
    feature 78 so b1 folds into W1.
  - Device (per core): L1 runs in float32r; the three HID x HID layers
    (L2/L3/L4) run as fp8(e4m3) DoubleRow matmuls (two 128-row contraction
    tiles per pass, 0.5 cycles/row on the PE) — weights and the
    inter-layer activations are quantized to e4m3, which keeps the final
    relative error ~1e-2 (measured against the fp64 reference on the
    actual problem inputs).  L2/L3 activations (relu+bias+fp8 convert) run
    on the scalar engine, L1 relu on DVE, L4 relu on gpsimd; the attention
    score dot-product is a single fused DVE tensor_tensor_reduce.  b4 is
    added via a tiny fp8 ones-row DoubleRow matmul into the same PSUM
    accumulation group.
  - Segment softmax pooling as small float32r PE matmuls E.T @ h4
    (E = A * e) accumulated into persistent PSUM across all chunks; the
    softmax denominator comes from E.T @ ones.  The final per-utterance
    MLP runs once at the end in float32r.
"""

import sys

sys.path.insert(0, "/opt/trn_rl_repo")

import numpy as np
import ml_dtypes

import concourse.bass as bass
import concourse.mybir as mybir
import concourse.tile as tile
from concourse import bacc
from concourse.bass_utils import run_bass_kernel_spmd

P = 128
FEAT = 78
HID = 1024
NCLS = 10
NSEG = 256
NCORES = 8
SEGS_PER_CORE = NSEG // NCORES
CH = 512           # frames per chunk (free dim of layer-1..3 matmuls)
FRT_PER_CH = CH // P
NCOL = 256         # moving-dim columns per DoubleRow matmul (HW limit)
KS = HID // P      # 8 k-subtiles
KP = KS // 2       # 4 DoubleRow k-pairs
F32 = mybir.dt.float32
F32R = mybir.dt.float32r
F8 = mybir.dt.float8e4
BF16 = mybir.dt.bfloat16
E4NP = ml_dtypes.float8_e4m3

# misc constant tile column layout ([128, 256] f32, host-packed)
MC_B2 = 0          # cols 0..7   : b2 striped [128, 8]
MC_B3 = 8          # cols 8..15  : b3 striped
MC_B5 = 17         # col 17      : b5 replicated down partitions
MC_ID = 128        # cols 128..159, rows 0..31: 32x32 identity
MC_ID2 = 160       # cols 160..191, rows 32..63: 32x32 identity (2nd block)
# f32r matmul-constants tile ([128, 128])
MM_ONES = 0        # cols 0..7   : ones columns (denom matmul rhs, N=8)
MM_W7 = 16         # cols 16..95 : W7 as [128, 8, 10]
# row constants tile ([1, 192] f32r, host-packed)
RW_ONES = 0        # cols 0..127 : ones row
RW_B7 = 128        # cols 128..137 : b7


def _segment_ids(lengths: np.ndarray, total: int) -> np.ndarray:
    """Replicate jnp.repeat(arange(n), lengths, total_repeat_length=total)."""
    lengths = np.asarray(lengths, dtype=np.int64)
    seg = np.repeat(np.arange(lengths.shape[0], dtype=np.int32), np.maximum(lengths, 0))
    if seg.shape[0] >= total:
        return seg[:total]
    pad_val = seg[-1] if seg.shape[0] > 0 else np.int32(0)
    return np.concatenate([seg, np.full(total - seg.shape[0], pad_val, np.int32)])


def _balance_segments(lengths: np.ndarray) -> list[list[int]]:
    """Assign 256 segments to 8 cores, 32 each, minimizing max frame count."""
    order = np.argsort(-lengths, kind="stable")
    loads = [0] * NCORES
    bins: list[list[int]] = [[] for _ in range(NCORES)]
    for s in order:
        cands = [c for c in range(NCORES) if len(bins[c]) < SEGS_PER_CORE]
        c = min(cands, key=lambda c: (loads[c], c))
        bins[c].append(int(s))
        loads[c] += int(lengths[s])
    for b in bins:
        b.sort()
    return bins


UNROLL = 16        # chunks per hardware-loop iteration


def _build_program(m_pad: int):
    """Emit the Bass/Tile program for one core with m_pad frames (static).

    Chunks 0 and nch-1 are peeled (they carry the PSUM accumulation
    start/stop flags); the middle chunks run in a Tile hardware loop
    (For_i) so per-engine semaphore counts reset every back-edge and the
    instruction stream stays small.
    """
    nch = m_pad // CH
    frt = m_pad // P
    S = SEGS_PER_CORE

    nc = bacc.Bacc("TRN2", target_bir_lowering=False, debug=False,
                   num_devices=NCORES)

    xT_d = nc.dram_tensor("xq", [P // 2, 2, m_pad], F8, kind="ExternalInput")
    A_d = nc.dram_tensor("Amat", [P, frt, S], F32, kind="ExternalInput")
    W1_d = nc.dram_tensor("W1q", [P // 2, 2, HID], F8, kind="ExternalInput")
    W2_d = nc.dram_tensor("W2q", [KP, P, 2, HID], F8, kind="ExternalInput")
    W3_d = nc.dram_tensor("W3q", [KP, P, 2, HID], F8, kind="ExternalInput")
    W4_d = nc.dram_tensor("W4q", [KP, P, 2, HID], F8, kind="ExternalInput")
    W5_d = nc.dram_tensor("W5rep", [P, HID], BF16, kind="ExternalInput")
    W6_d = nc.dram_tensor("W6", [HID, HID], F32R, kind="ExternalInput")
    b4_d = nc.dram_tensor("b4q", [1, 2, HID], F8, kind="ExternalInput")
    on_d = nc.dram_tensor("onesq", [1, 2, P], F8, kind="ExternalInput")
    b6_d = nc.dram_tensor("b6r", [1, HID], F32R, kind="ExternalInput")
    misc_d = nc.dram_tensor("miscc", [P, 256], F32, kind="ExternalInput")
    mmc_d = nc.dram_tensor("mmcc", [P, P], F32R, kind="ExternalInput")
    row_d = nc.dram_tensor("rowm", [1, 192], F32R, kind="ExternalInput")
    out_d = nc.dram_tensor("out", [S, NCLS], F32, kind="ExternalOutput")

    RELU = mybir.ActivationFunctionType.Relu
    EXP = mybir.ActivationFunctionType.Exp
    MULT = mybir.AluOpType.mult
    ADD = mybir.AluOpType.add
    MAX = mybir.AluOpType.max
    DR = mybir.MatmulPerfMode.DoubleRow

    with tile.TileContext(nc) as tc:
        with (
            tc.tile_pool(name="wpool", bufs=1) as wpool,
            tc.tile_pool(name="xpool", bufs=2) as xpool,
            tc.tile_pool(name="apool", bufs=2) as apool,
            tc.tile_pool(name="hpool", bufs=2) as hpool,
            tc.tile_pool(name="h4pool", bufs=12) as h4pool,
            tc.tile_pool(name="spool", bufs=4) as spool,
            tc.tile_pool(name="colpool", bufs=8) as colpool,
            tc.tile_pool(name="epool", bufs=8) as epool,
            tc.tile_pool(name="psA", bufs=4, space="PSUM") as psA,
            tc.tile_pool(name="psB", bufs=2, space="PSUM") as psB,
            tc.tile_pool(name="psAcc", bufs=1, space="PSUM") as psAcc,
        ):
            # ---- resident constants/weights ----
            # Small tensors needed by chunk 0's L1 go on the DMA queue
            # first; the 3MB of fp8 layer weights are issued AFTER chunk 0's
            # x/A DMA (via the preload callback) so L1 can start within ~2us
            # and L2/L3/L4 weights stream in behind it.
            W1s = wpool.tile([P // 2, 2, HID], F8, tag="W1")
            nc.sync.dma_start(W1s[:], W1_d.ap())
            misc = wpool.tile([P, 256], F32, tag="misc")
            nc.sync.dma_start(misc[:], misc_d.ap())
            mmc = wpool.tile([P, P], F32R, tag="mmc")
            nc.sync.dma_start(mmc[:], mmc_d.ap())
            rowm = wpool.tile([1, 192], F32R, tag="rowm")
            nc.sync.dma_start(rowm[:], row_d.ap())
            b4qs = wpool.tile([1, 2, HID], F8, tag="b4q")
            nc.sync.dma_start(b4qs[:], b4_d.ap())
            onesq = wpool.tile([1, 2, P], F8, tag="onesq")
            nc.sync.dma_start(onesq[:], on_d.ap())
            b6s = wpool.tile([1, HID], F32R, tag="b6")
            nc.sync.dma_start(b6s[:], b6_d.ap())

            W2q = [wpool.tile([P, 2, HID], F8, tag=f"W2q{j}", name=f"W2q{j}")
                   for j in range(KP)]
            W3q = [wpool.tile([P, 2, HID], F8, tag=f"W3q{j}", name=f"W3q{j}")
                   for j in range(KP)]
            W4q = [wpool.tile([P, 2, HID], F8, tag=f"W4q{j}", name=f"W4q{j}")
                   for j in range(KP)]
            W5s = wpool.tile([P, HID], BF16, tag="W5")

            def load_big_weights():
                for d, tiles in ((W2_d, W2q), (W3_d, W3q), (W4_d, W4q)):
                    for j in range(KP):
                        nc.sync.dma_start(tiles[j][:], d.ap()[j])
                nc.sync.dma_start(W5s[:], W5_d.ap())

            ones_row = rowm[:, RW_ONES:RW_ONES + P]
            ones_col = mmc[:, MM_ONES:MM_ONES + 8]
            b5col = misc[:, MC_B5:MC_B5 + 1]
            ident = misc[:S, MC_ID:MC_ID + S]
            W7v = mmc[:, MM_W7:MM_W7 + KS * NCLS].rearrange(
                "p (o c) -> p o c", c=NCLS)
            b7row = rowm[:, RW_B7:RW_B7 + NCLS]

            # persistent PSUM accumulators (own banks for the whole pass);
            # the softmax denominator is accumulated in SBUF by the DVE
            # (eacc += et) and partition-reduced once in the tail, which
            # frees a PSUM bank for a 4th psA buffer
            pooled0 = psAcc.tile([S, 512], F32, tag="pooled0")
            pooled1 = psAcc.tile([S, 512], F32, tag="pooled1")
            # eacc must be f32r-typed (it feeds an f32r matmul in the tail;
            # walrus requires producers of f32r-matmul operands to round);
            # instead of a memset, the very first accumulation is a copy
            eacc = wpool.tile([P, S], F32R, tag="eacc")

            # ---- main pass over frame chunks ----
            # Pooling matmuls are deferred by one chunk (emitted after the
            # NEXT chunk's L3) so the PE never stalls on the cross-engine
            # score chain (gpsimd relu -> DVE reduce -> scalar exp -> DVE
            # mul) that produces et.
            pend: list[tuple] = []

            def flush_pool():
                while pend:
                    et, h4, st, sp = pend.pop(0)
                    nc.tensor.matmul(pooled0[:], et[:], h4[:, :512],
                                     start=st, stop=sp)
                    nc.tensor.matmul(pooled1[:], et[:], h4[:, 512:],
                                     start=st, stop=sp)

            def chunk_group(c0, n_chunks, first=False, last=False,
                            preload=None):
                """Emit n_chunks chunks starting at chunk index c0 (int or
                loop ScalarValue). first/last carry PSUM group flags."""
                xg = xpool.tile([P // 2, 2, UNROLL * CH], F8, tag="x")
                nc.sync.dma_start(
                    xg[:, :, :n_chunks * CH],
                    xT_d.ap()[:, :, bass.ds(c0 * CH, n_chunks * CH)])
                ag = apool.tile([P, UNROLL * FRT_PER_CH, S], F32, tag="A")
                nc.sync.dma_start(
                    ag[:, :n_chunks * FRT_PER_CH, :],
                    A_d.ap()[:, bass.ds(c0 * FRT_PER_CH,
                                        n_chunks * FRT_PER_CH), :])
                if preload is not None:
                    preload()

                # --- per-chunk layer emitters (skewed pipeline below) ---
                h1s: dict[int, object] = {}
                h2s: dict[int, object] = {}
                h3s: dict[int, object] = {}

                def do_L1(u):
                    # fp8 DoubleRow over two 64-feature halves (b1 folded
                    # via ones feature); relu on DVE + gpsimd
                    h1 = hpool.tile([P, KS, CH], F8, tag="h1")
                    for m in range(KS):
                        ps = psA.tile([P, CH], F32, tag="mm")
                        for c2 in range(CH // NCOL):
                            xt = xg[:, :, u * CH + c2 * NCOL:
                                    u * CH + (c2 + 1) * NCOL]
                            nc.tensor.matmul(ps[:, c2 * NCOL:(c2 + 1) * NCOL],
                                             W1s[:, :, m * P:(m + 1) * P],
                                             xt, start=True, stop=True,
                                             perf_mode=DR)
                        nc.vector.tensor_scalar_max(h1[:, m, :], ps[:], 0.0)
                    h1s[u] = h1

                def do_L23(u, Wq, boff, li):
                    # fp8 DoubleRow; relu+bias on scalar (L3 m<2 on DVE)
                    h_in = h1s.pop(u) if li == 2 else h2s.pop(u)
                    h_out = hpool.tile([P, KS, CH], F8,
                                       tag=("h2" if li == 2 else "h3"))
                    for m in range(KS):
                        ps = psA.tile([P, CH], F32, tag="mm")
                        for c2 in range(CH // NCOL):
                            seg = ps[:, c2 * NCOL:(c2 + 1) * NCOL]
                            for j in range(KP):
                                nc.tensor.matmul(
                                    seg, Wq[j][:, :, m * P:(m + 1) * P],
                                    h_in[:, 2 * j:2 * j + 2,
                                         c2 * NCOL:(c2 + 1) * NCOL],
                                    start=(j == 0), stop=(j == KP - 1),
                                    perf_mode=DR)
                        bcol = misc[:, boff + m:boff + m + 1]
                        if m >= 6:
                            nc.vector.tensor_scalar(
                                out=h_out[:, m, :], in0=ps[:],
                                scalar1=bcol, scalar2=0.0,
                                op0=ADD, op1=MAX)
                        else:
                            nc.scalar.activation(
                                h_out[:, m, :], ps[:], RELU, bias=bcol)
                    (h2s if li == 2 else h3s)[u] = h_out

                h4s: dict[int, list] = {}

                def do_L4(u):
                    # frame-major fp8 DR; relu on DVE (n=0) / scalar (n=1);
                    # h4 is bf16 so the score reduction runs at DVE 2x rate
                    # and pooling matmuls stay 1 cycle/row
                    h3 = h3s.pop(u)
                    tiles = []
                    for f in range(FRT_PER_CH):
                        h4 = h4pool.tile([P, HID], BF16, tag="h4")
                        for n in range(2):
                            ps4 = psB.tile([P, 512], F32, tag="l4")
                            for c2 in range(2):
                                seg = ps4[:, c2 * 256:(c2 + 1) * 256]
                                col0 = n * 512 + c2 * 256
                                for j in range(KP):
                                    nc.tensor.matmul(
                                        seg,
                                        h3[:, 2 * j:2 * j + 2, f * P:(f + 1) * P],
                                        W4q[j][:, :, col0:col0 + 256],
                                        start=(j == 0), stop=False,
                                        perf_mode=DR)
                                nc.tensor.matmul(
                                    seg, onesq[:],
                                    b4qs[:, :, col0:col0 + 256],
                                    start=False, stop=True, perf_mode=DR)
                            nc.scalar.activation(h4[:, n * 512:(n + 1) * 512],
                                                 ps4[:], RELU)
                        tiles.append(h4)
                    h4s[u] = tiles

                def do_scores(u):
                    # d = sum(h4*W5rep); e = max(exp(d + b5), 1); E = A*e.
                    # tensor_tensor_reduce crashes this HW build, so the
                    # reduction runs as gpsimd multiply (the Pool engine is
                    # otherwise idle) + DVE tensor_scalar with accum_out.
                    for f, h4 in enumerate(h4s.pop(u)):
                        prod = spool.tile([P, HID], BF16, tag="sc")
                        ct = colpool.tile([P, 16], F32, tag="col")
                        nc.vector.tensor_tensor(
                            out=prod[:], in0=h4[:], in1=W5s[:], op=MULT)
                        nc.vector.tensor_scalar(
                            out=prod[:], in0=prod[:], scalar1=1.0,
                            scalar2=0.0, op0=MULT, op1=ADD,
                            accum_out=ct[:, 0:1])
                        nc.scalar.activation(ct[:, 1:2], ct[:, 0:1], EXP,
                                             bias=b5col)
                        nc.gpsimd.tensor_scalar_max(ct[:, 2:3], ct[:, 1:2], 1.0)
                        et = epool.tile([P, S], BF16, tag="E")
                        nc.gpsimd.tensor_scalar_mul(
                            et[:], ag[:, u * FRT_PER_CH + f, :], ct[:, 2:3])
                        if first and u == 0 and f == 0:
                            nc.vector.tensor_copy(out=eacc[:], in_=et[:])
                        else:
                            nc.vector.tensor_tensor(
                                out=eacc[:], in0=eacc.bitcast(F32)[:],
                                in1=et[:], op=ADD)

                        st = bool(first and u == 0 and f == 0)
                        sp = bool(last and u == n_chunks - 1
                                  and f == FRT_PER_CH - 1)
                        pend.append((et, h4, st, sp))

                # --- 4-deep skewed pipeline over this group's chunks: at
                # step s the PE runs L2(s-1), L3(s-2), L1(s), L4(s-3), so
                # every matmul's inputs were activated a full step (~13us
                # of PE work) earlier and no act->matmul chain is on the
                # critical path.  Layer order within the step is chosen so
                # each layer's psA buffers were last touched by acts that
                # are already 1+ layers old.  The score chain trails L4 by
                # a step and pooling trails scores by one more.
                for s in range(n_chunks + 5):
                    if 0 <= s - 1 < n_chunks:
                        do_L23(s - 1, W2q, MC_B2, 2)
                    if 0 <= s - 2 < n_chunks:
                        do_L23(s - 2, W3q, MC_B3, 3)
                    if s < n_chunks:
                        do_L1(s)
                    flush_pool()
                    if 0 <= s - 3 < n_chunks:
                        do_L4(s - 3)
                    if 0 <= s - 4 < n_chunks:
                        do_scores(s - 4)

                # pend MUST be empty at every chunk_group return: entries may
                # not cross a For_i trace boundary (buffer rotation would
                # resolve to the wrong copies at runtime, and chunk 0's
                # start=True matmul must stay outside the loop body)
                flush_pool()

            # peel chunk 0 (PSUM group start) and chunk nch-1 (stop)
            import os
            chunk_group(0, 1, first=True, last=(nch == 1),
                        preload=load_big_weights)
            if nch == 1:
                pass
            elif os.environ.get("KERNEL_STATIC_UNROLL"):
                # cost-model twin: same stream, no dynamic loop machinery
                c = 1
                while c < nch - 1:
                    n = min(UNROLL, nch - 1 - c)
                    chunk_group(c, n)
                    c += n
            elif nch > 2:
                # 8 chunks per back-edge
                def loop_body(iv, unroll):
                    off = 0
                    while off < unroll:
                        n = min(UNROLL, unroll - off)
                        chunk_group(iv + off, n)
                        off += n
                tc.For_i_unrolled_general(
                    start=1, end=nch - 1, step=1,
                    unrollable_body=loop_body,
                    max_unroll=UNROLL,
                    hint_engines=(mybir.EngineType.PE,),
                )
            if nch > 1:
                chunk_group(nch - 1, 1, last=True)

            # W6 load: emitted here but the DMA engine runs well ahead of
            # the PE (its issue is only throttled by the x/A prefetch
            # queue), so the 4MB is resident long before the tail needs it
            W6s = []
            for k in range(KS):
                t = wpool.tile([P, HID], F32R, tag=f"W6k{k}")
                nc.sync.dma_start(t[:], W6_d.ap()[k * P:(k + 1) * P, :])
                W6s.append(t)

            # ---- final per-utterance MLP (f32r) ----
            # denom[seg] = sum_p eacc[p, seg] via one tiny PE matmul
            psd = psA.tile([S, 8], F32, tag="mm")
            nc.tensor.matmul(psd[:], eacc[:], ones_col,
                             start=True, stop=True)
            fc = colpool.tile([P, 16], F32, tag="col")
            nc.vector.tensor_copy(out=fc[:S, 0:1], in_=psd[:, 0:1])
            nc.vector.reciprocal(fc[:S, 1:2], fc[:S, 0:1])

            pooled_sb = spool.tile([S, HID], F32, tag="sc")
            nc.vector.tensor_scalar_mul(pooled_sb[:, :512], pooled0[:],
                                        fc[:S, 1:2])
            nc.vector.tensor_scalar_mul(pooled_sb[:, 512:], pooled1[:],
                                        fc[:S, 1:2])

            # transpose pooled -> pooledT [hid, seg]
            tposed = wpool.tile([P, KS, 2 * S], F32R, tag="tposed")
            pooledT = tposed[:, :, :S]
            gT = tposed[:, :, S:]
            for k in range(KS):
                pst = psA.tile([P, S], F32, tag="mm")
                nc.tensor.transpose(pst[:], pooled_sb[:, k * P:(k + 1) * P],
                                    ident)
                nc.vector.tensor_copy(out=pooledT[:, k, :], in_=pst[:])

            # g = relu(pooled @ W6 + b6)   (seg-major [S, HID])
            g_sb = spool.tile([S, HID], F32, tag="sc")
            for n in range(2):
                psg = psB.tile([S, 512], F32, tag="l4")
                for k in range(KS):
                    nc.tensor.matmul(psg[:], pooledT[:, k, :],
                                     W6s[k][:, n * 512:(n + 1) * 512],
                                     start=(k == 0), stop=False)
                nc.tensor.matmul(psg[:], ones_row[:, :S],
                                 b6s[:, n * 512:(n + 1) * 512],
                                 start=False, stop=True)
                nc.scalar.activation(g_sb[:, n * 512:(n + 1) * 512], psg[:], RELU)

            # gT [hid, seg]
            for k in range(KS):
                pst = psA.tile([P, S], F32, tag="mm")
                nc.tensor.transpose(pst[:], g_sb[:, k * P:(k + 1) * P], ident)
                nc.vector.tensor_copy(out=gT[:, k, :], in_=pst[:])

            # out = g @ W7 + b7
            pso = psA.tile([S, NCLS], F32, tag="mm")
            for k in range(KS):
                nc.tensor.matmul(pso[:], gT[:, k, :], W7v[:, k, :],
                                 start=(k == 0), stop=False)
            nc.tensor.matmul(pso[:], ones_row[:, :S], b7row,
                             start=False, stop=True)
            oc = colpool.tile([S, 16], F32, tag="col")
            nc.vector.tensor_copy(out=oc[:, :NCLS], in_=pso[:])
            nc.sync.dma_start(out_d.ap()[:], oc[:, :NCLS])

    nc.compile()
    return nc


def _q8(a: np.ndarray) -> np.ndarray:
    return np.asarray(a, dtype=np.float32).astype(E4NP)


def _pack_dr(W: np.ndarray) -> np.ndarray:
    """[1024, N] weight matrix -> DoubleRow fp8 layout [KP, 128, 2, N]."""
    return np.ascontiguousarray(
        _q8(W).reshape(KP, 2, P, -1).transpose(0, 2, 1, 3))


def prepare_inputs(x, W1, b1, W2, b2, W3, b3, W4, b4, W5, b5, W6, b6, W7, b7,
                   lengths):
    """Host-side sharding/packing. Returns (in_maps, bins, m_pad)."""
    x = np.ascontiguousarray(np.asarray(x, dtype=np.float32))
    lengths = np.asarray(lengths)
    total = x.shape[0]
    seg_ids = _segment_ids(lengths, total)
    counts = np.bincount(seg_ids, minlength=NSEG).astype(np.int64)
    starts = np.zeros(NSEG + 1, dtype=np.int64)
    starts[1:] = np.cumsum(counts)

    bins = _balance_segments(counts)
    core_frames = [int(sum(counts[s] for s in b)) for b in bins]
    m_pad = ((max(core_frames) + CH - 1) // CH) * CH
    frt = m_pad // P

    W1p = np.zeros((P, HID), dtype=np.float32)
    W1p[:FEAT] = np.asarray(W1, dtype=np.float32)
    W1p[FEAT] = np.asarray(b1, dtype=np.float32)
    # DoubleRow over two 64-feature halves: [64, 2, HID]
    W1q = np.ascontiguousarray(
        _q8(W1p).reshape(2, P // 2, HID).transpose(1, 0, 2))

    misc = np.zeros((P, 256), dtype=np.float32)
    misc[:, MC_B2:MC_B2 + KS] = np.asarray(b2, np.float32).reshape(KS, P).T
    misc[:, MC_B3:MC_B3 + KS] = np.asarray(b3, np.float32).reshape(KS, P).T
    misc[:, MC_B5] = np.float32(np.asarray(b5, np.float32).reshape(-1)[0])
    misc[:SEGS_PER_CORE, MC_ID:MC_ID + SEGS_PER_CORE] = np.eye(
        SEGS_PER_CORE, dtype=np.float32)
    misc[SEGS_PER_CORE:2 * SEGS_PER_CORE, MC_ID2:MC_ID2 + SEGS_PER_CORE] = \
        np.eye(SEGS_PER_CORE, dtype=np.float32)

    mmcc = np.zeros((P, P), dtype=np.float32)
    mmcc[:, MM_ONES:MM_ONES + 8] = 1.0
    mmcc[:, MM_W7:MM_W7 + KS * NCLS] = np.asarray(W7, np.float32).reshape(
        KS, P, NCLS).transpose(1, 0, 2).reshape(P, KS * NCLS)

    rowm = np.zeros((1, 192), dtype=np.float32)
    rowm[0, RW_ONES:RW_ONES + P] = 1.0
    rowm[0, RW_B7:RW_B7 + NCLS] = np.asarray(b7, np.float32).reshape(-1)

    b4q = np.zeros((1, 2, HID), dtype=E4NP)
    b4q[0, 0, :] = _q8(np.asarray(b4, np.float32).reshape(-1))
    onesq = np.zeros((1, 2, P), dtype=E4NP)
    onesq[0, 0, :] = np.float32(1.0)

    shared = dict(
        W1q=W1q,
        W2q=_pack_dr(np.asarray(W2, np.float32)),
        W3q=_pack_dr(np.asarray(W3, np.float32)),
        W4q=_pack_dr(np.asarray(W4, np.float32)),
        W5rep=np.broadcast_to(
            np.asarray(W5, np.float32).reshape(1, HID).astype(
                ml_dtypes.bfloat16), (P, HID)).copy(),
        W6=np.ascontiguousarray(np.asarray(W6, np.float32)),
        b4q=b4q,
        onesq=onesq,
        b6r=np.asarray(b6, np.float32).reshape(1, HID),
        miscc=misc,
        mmcc=mmcc,
        rowm=rowm,
    )

    in_maps = []
    for core in range(NCORES):
        segs = bins[core]
        xs = [x[starts[s]:starts[s + 1]] for s in segs]
        xcat = np.concatenate(xs, axis=0) if xs else np.zeros((0, FEAT), np.float32)
        n = xcat.shape[0]
        xT = np.zeros((P, m_pad), dtype=np.float32)
        xT[:FEAT, :n] = xcat.T
        xT[FEAT, :n] = 1.0  # constant feature -> b1
        A = np.zeros((m_pad, SEGS_PER_CORE), dtype=np.float32)
        off = 0
        for j, s in enumerate(segs):
            ln = int(counts[s])
            A[off:off + ln, j] = 1.0
            off += ln
        im = dict(shared)
        # fp8 x, DoubleRow halves: xq[p, i, col] = xpad[i*64 + p, col]
        im["xq"] = np.ascontiguousarray(
            _q8(xT).reshape(2, P // 2, m_pad).transpose(1, 0, 2))
        # partition-major layout [P, frt, S]: Ah[p, t, s] = A[t*128 + p, s]
        im["Amat"] = np.ascontiguousarray(
            A.reshape(frt, P, SEGS_PER_CORE).transpose(1, 0, 2))
        in_maps.append(im)
    return in_maps, bins, m_pad


_PROGRAM_CACHE: dict[int, object] = {}


def kernel(**inputs) -> np.ndarray:
    in_maps, bins, m_pad = prepare_inputs(**inputs)
    nc = _PROGRAM_CACHE.get(m_pad)
    if nc is None:
        nc = _build_program(m_pad)
        _PROGRAM_CACHE[m_pad] = nc
    res = run_bass_kernel_spmd(nc, in_maps, core_ids=list(range(NCORES)))
    out = np.zeros((NSEG, NCLS), dtype=np.float32)
    for core in range(NCORES):
        out[bins[core]] = res.results[core]["out"]
    return out
